# revision 2
# baseline (speedup 1.0000x reference)
"""Trainium2 Bass kernel for: conv2d(16->64, 3x3, VALID) + bias -> min over
channels -> tanh(tanh()).  Input x [64,16,256,256] f32, output [64,1,254,254].

Strategy (per core, data-parallel over batch: 8 images/core):
  - The conv is computed as matmuls with the *x-patch as the stationary
    operand* and a block-Toeplitz weight matrix as the moving operand, so the
    conv output lands as [width-positions (partitions), rows*couts (free)].
    That makes the channel-min a cheap free-dim DVE reduce_min.
  - Contraction K = 17 channels x 7 rows = 119 (channel 16 is a host-added
    ones-channel that carries the bias via an extra weight row).
  - A 7-row input window yields R=5 output rows per matmul group; the 3
    kernel x-taps (dx) are 3 PSUM-accumulated matmuls whose stationary
    operands are the same SBUF window tile sliced at column offset dx.
  - min over couts: DVE tensor_reduce(min) over the innermost 64-wide axis
    of the [127, 5, 64] PSUM view -> [127 positions, 5 rows].
  - Results accumulate in an SBUF staging tile [127 j, 254 rows]; PE
    transposes 127x127 chunks; double-tanh on ScalarE; DMA out.
"""

import sys

for _p in ("/opt/trn_rl_repo", "/root/.axon_site/_ro/trn_rl_repo"):
    if _p not in sys.path:
        sys.path.insert(0, _p)

import numpy as np

B, CIN, H, W = 64, 16, 256, 256
COUT, KK = 64, 3
HO, WO = H - 2, W - 2  # 254
N_CORES = 8
B_LOC = B // N_CORES  # 8 images per core

# geometry
WIN_ROWS = 7          # input rows per window
R = WIN_ROWS - KK + 1  # 5 output rows per window
KDIM = (CIN + 1) * WIN_ROWS  # 119 contraction rows (incl. ones channel)
NDIM = R * COUT       # 320 moving free size
MJ = 127              # output width positions per j-block
N_JB = 2              # j blocks (2*127 = 254)
N_WIN = 51            # windows: row0 = 5w for w<50, 249 for w=50
WIN_COLS = MJ + KK - 1  # 129 columns per window tile

_cache = {}


def _build_wblocks(conv_weight, conv_bias):
    """wblk[dx][rho*17+ci, r*64+co] = W[co,ci,rho-r,dx]; bias on the ones-
    channel row (rho=0, ci=CIN) of dx=0.  Partition order matches the
    [B, H, C, W] host layout of x so the window DMA merges (row, chan)."""
    wblk = np.zeros((KK, KDIM, NDIM), dtype=np.float32)
    for dx in range(KK):
        for ci in range(CIN):
            for rho in range(WIN_ROWS):
                k = rho * (CIN + 1) + ci
                for r in range(R):
                    dy = rho - r
                    if 0 <= dy < KK:
                        wblk[dx, k, r * COUT:(r + 1) * COUT] = conv_weight[:, ci, dy, dx]
    k_bias = CIN  # (rho=0, ci=16)
    for r in range(R):
        wblk[0, k_bias, r * COUT:(r + 1) * COUT] = conv_bias
    return wblk


def _build_nc(reps=1, ablate=()):
    import concourse.bass as bass
    import concourse.bacc as bacc
    import concourse.tile as tile
    from concourse import mybir

    f32 = mybir.dt.float32
    f32r = mybir.dt.float32r

    nc = bacc.Bacc(None)
    # x_aug host layout is [B, H, C, W]: window partitions are (row, chan)
    x_aug = nc.dram_tensor("x_aug", [B_LOC, H, CIN + 1, W], f32r, kind="ExternalInput")
    wblk_d = nc.dram_tensor("wblk", [KK, KDIM, NDIM], f32r, kind="ExternalInput")
    ident_d = nc.dram_tensor("ident", [MJ, MJ], f32, kind="ExternalInput")
    y = nc.dram_tensor("y", [B_LOC, HO, WO], f32, kind="ExternalOutput")

    with tile.TileContext(nc) as tc:
        with (
            tc.tile_pool(name="consts", bufs=1) as consts,
            tc.tile_pool(name="wins", bufs=3) as wins,
            tc.tile_pool(name="stage", bufs=4) as stage,
            tc.tile_pool(name="outs", bufs=4) as outs,
            tc.tile_pool(name="cpsum", bufs=6, space="PSUM") as cpsum,
            tc.tile_pool(name="tpsum", bufs=2, space="PSUM") as tpsum,
        ):
            wblk_s = consts.tile([KDIM, KK, NDIM], f32r)
            nc.sync.dma_start(out=wblk_s[:], in_=wblk_d.rearrange("k d n -> d k n"))
            ident_s = consts.tile([MJ, MJ], f32)
            nc.sync.dma_start(out=ident_s[:], in_=ident_d[:])

            wt0 = consts.tile([KDIM, WIN_COLS], f32r)
            nc.sync.dma_start(
                out=wt0[:],
                in_=x_aug[0, 0:WIN_ROWS, :, 0:WIN_COLS].rearrange("r c w -> (r c) w"),
            )
            psum0 = cpsum.tile([MJ, NDIM], f32, bufs=1) if "nomm" in ablate else None
            if psum0 is not None:
                nc.tensor.matmul(out=psum0[:], lhsT=wt0[:, 0:MJ],
                                 rhs=wblk_s[:, 0, :], start=True, stop=True)

            import contextlib
            loop_ctx = tc.For_i(0, reps, 1) if reps > 1 else contextlib.nullcontext()
            with loop_ctx:
                _emit_body(nc, tc, bass, mybir, ablate, locals())
    nc.finalize()
    return nc


def _emit_body(nc, tc, bass, mybir, ablate, env):
    f32 = env["f32"]
    f32r = env["f32r"]
    x_aug, y = env["x_aug"], env["y"]
    wblk_s, ident_s = env["wblk_s"], env["ident_s"]
    wins, stage, outs = env["wins"], env["stage"], env["outs"]
    cpsum, tpsum = env["cpsum"], env["tpsum"]
    psum0 = env["psum0"]
    CW = (CIN + 1) * W  # elements per image row (all channels)

    for b in range(B_LOC):
        if "nodma" in ablate:
            bigx = wins.tile([KDIM, 1, W], f32r, name="bigx")
            nc.sync.dma_start(
                out=bigx[:, 0, :],
                in_=x_aug[b, 0:WIN_ROWS, :, :].rearrange("r c w -> (r c) w"),
            )
        else:
            bigx = wins.tile([KDIM, N_WIN, W], f32r, name="bigx")
        if "nodma" not in ablate:
            # windows 0..49 (uniform row0 = 5w) in 4 chunked DMAs; w=50 alone
            x_b = x_aug[b]
            for ci, w_lo in enumerate(range(0, 50, 13)):
                w_hi = min(w_lo + 13, 50)
                nw = w_hi - w_lo
                src = bass.AP(
                    tensor=x_b.tensor,
                    offset=x_b.offset + 5 * w_lo * CW,
                    ap=[[CW, WIN_ROWS], [W, CIN + 1], [5 * CW, nw], [1, W]],
                )
                nc.sync.dma_start(out=bigx[:, w_lo:w_hi, :], in_=src)
            nc.sync.dma_start(
                out=bigx[:, N_WIN - 1, :],
                in_=x_aug[b, HO - R:H, :, :].rearrange("r c w -> (r c) w"),
            )
        stagings = []
        for jb in range(N_JB):
            staging = stage.tile([MJ, 256], f32, name=f"staging{jb}", tag=f"st{jb}")
            stagings.append(staging)
            if "nodve" in ablate:
                nc.vector.memset(staging[:], 0.0)
        for w in range(N_WIN):
            row0 = 5 * w if w < N_WIN - 1 else HO - R
            for jb in range(N_JB):
                j0 = jb * MJ
                if "nomm" in ablate:
                    psum = psum0
                else:
                    psum = cpsum.tile([MJ, NDIM], f32, name="psum")
                    wi = 0 if "nodma" in ablate else w
                    for dx in range(KK):
                        nc.tensor.matmul(
                            out=psum[:],
                            lhsT=bigx[:, wi, j0 + dx:j0 + dx + MJ],
                            rhs=wblk_s[:, dx, :],
                            start=(dx == 0),
                            stop=(dx == KK - 1),
                        )
                if "nodve" not in ablate:
                    nc.vector.tensor_reduce(
                        out=stagings[jb][:, row0:row0 + R],
                        in_=psum.rearrange("p (r c) -> p r c", c=COUT),
                        axis=mybir.AxisListType.X,
                        op=mybir.AluOpType.min,
                    )
        if "noepi" in ablate:
            for jb in range(N_JB):
                nc.sync.dma_start(out=y[b, 0:MJ, jb * MJ:jb * MJ + MJ],
                                  in_=stagings[jb][:, 0:MJ])
            continue
        for rb in range(2):
            r0 = rb * MJ
            t2 = outs.tile([MJ, N_JB, MJ], f32, name="t2")
            for jb in range(N_JB):
                ps_t = tpsum.tile([MJ, MJ], f32, name="ps_t")
                nc.tensor.transpose(
                    out=ps_t[:], in_=stagings[jb][:, r0:r0 + MJ],
                    identity=ident_s[:],
                )
                t1 = outs.tile([MJ, MJ], f32, name="t1")
                nc.scalar.activation(
                    out=t1[:], in_=ps_t[:],
                    func=mybir.ActivationFunctionType.Tanh,
                )
                nc.scalar.activation(
                    out=t2[:, jb, :], in_=t1[:],
                    func=mybir.ActivationFunctionType.Tanh,
                )
            # one wide store per row-block on the ACT HWDGE ring (parallel
            # to the input ring); per-partition run = 2*127*4B contiguous-ish
            nc.sync.dma_start(
                out=y[b, r0:r0 + MJ, :], in_=t2.rearrange("p a w -> p (a w)")
            )


def _get_compiled(reps=1, ablate=()):
    key = ("nc", reps, tuple(ablate))
    if key not in _cache:
        _cache[key] = _build_nc(reps, ablate)
    return _cache[key]


def build_in_maps(inputs):
    x = np.asarray(inputs["x"], dtype=np.float32)
    conv_weight = np.asarray(inputs["conv_weight"], dtype=np.float32)
    conv_bias = np.asarray(inputs["conv_bias"], dtype=np.float32)

    x_aug = np.empty((B, H, CIN + 1, W), dtype=np.float32)
    x_aug[:, :, :CIN] = x.transpose(0, 2, 1, 3)
    x_aug[:, :, CIN] = 1.0
    wblk = _build_wblocks(conv_weight, conv_bias)
    ident = np.eye(MJ, dtype=np.float32)

    return [
        {
            "x_aug": np.ascontiguousarray(x_aug[c * B_LOC:(c + 1) * B_LOC]),
            "wblk": wblk,
            "ident": ident,
        }
        for c in range(N_CORES)
    ]


def kernel(x, conv_weight, conv_bias):
    from concourse.bass_utils import run_bass_kernel_spmd

    in_maps = build_in_maps(
        {"x": x, "conv_weight": conv_weight, "conv_bias": conv_bias})
    nc = _get_compiled()
    res = run_bass_kernel_spmd(nc, in_maps, core_ids=list(range(N_CORES)))
    out = np.concatenate([res.results[c]["y"] for c in range(N_CORES)], axis=0)
    return out.reshape(B, 1, HO, WO)



# revision 4
# speedup vs baseline: 1.2248x; 1.2248x over previous
"""Trainium2 Bass kernel for: conv2d(16->64, 3x3, VALID) + bias -> min over
channels -> tanh(tanh()).  Input x [64,16,256,256] f32, output [64,1,254,254].

Strategy (per core, data-parallel over batch: 8 images/core):
  - Conv as matmuls with the *x-patch as the stationary operand* (bf16) and a
    block-Toeplitz weight matrix as the moving operand, so conv output lands
    as [width-positions (partitions), rows*couts (free)] -> channel-min is a
    free-dim DVE reduce_min.
  - Contraction K = 17 channels x 7 rows = 119 (channel 16 is a host-added
    ones-channel carrying the bias).  A 7-row window yields R=5 output rows;
    the 3 kernel x-taps are 3 PSUM-accumulated matmuls whose stationary
    operands are the same SBUF window tile sliced at column offset dx.
  - bf16 stationaries are 128 columns wide -> compiler enables FWL (2x faster
    LDWEIGHTS, hidden under the 320-col matmuls).
  - DVE reduce_min batches THREE windows per instruction (psum groups at
    512-elem bank strides) to amortize the 120-cycle PSUM-source overhead:
    in [128, 3, 5, 64] -> out [128, 15] staging rows.
  - Epilogue: PE transposes 128x128 chunks of the [j, row] staging; double
    tanh on ScalarE; DMA out.  j/row blocks at offsets {0, 126} overlap by 2
    (identical values) to cover 254 with 128-wide tiles.
"""

import sys

for _p in ("/opt/trn_rl_repo", "/root/.axon_site/_ro/trn_rl_repo"):
    if _p not in sys.path:
        sys.path.insert(0, _p)

import numpy as np

B, CIN, H, W = 64, 16, 256, 256
COUT, KK = 64, 3
HO, WO = H - 2, W - 2  # 254
N_CORES = 8
B_LOC = B // N_CORES  # 8 images per core

# geometry
WIN_ROWS = 7           # input rows per window
R = WIN_ROWS - KK + 1  # 5 output rows per window
KDIM = (CIN + 1) * WIN_ROWS  # 119 contraction rows (incl. ones channel)
NDIM = R * COUT        # 320 moving free size
MJ = 128               # output width positions per j-block (FWL needs 128)
JOFF = (0, WO - MJ)    # j-block origins: {0, 126}, overlap 2 cols
N_JB = 2
N_WIN = 51             # windows: row0 = 5w for w<50, 249 for w=50
GRP = 3                # windows per DVE reduce
PS_STRIDE = 512        # f32 elems between psum groups (= one 2KB bank)

_cache = {}


def _build_wblocks(conv_weight, conv_bias):
    """wblk[dx][rho*17+ci, r*64+co] = W[co,ci,rho-r,dx]; bias on the ones-
    channel row (rho=0, ci=CIN) of dx=0.  Partition order matches the
    [B, H, C, W] host layout of x so the window DMA merges (row, chan)."""
    wblk = np.zeros((KK, KDIM, NDIM), dtype=np.float32)
    for dx in range(KK):
        for ci in range(CIN):
            for rho in range(WIN_ROWS):
                k = rho * (CIN + 1) + ci
                for r in range(R):
                    dy = rho - r
                    if 0 <= dy < KK:
                        wblk[dx, k, r * COUT:(r + 1) * COUT] = conv_weight[:, ci, dy, dx]
    k_bias = CIN  # (rho=0, ci=16)
    for r in range(R):
        wblk[0, k_bias, r * COUT:(r + 1) * COUT] = conv_bias
    return wblk


def _win_groups():
    """Reduce groups: 16 triples (w 0..47), the pair (48,49), single (50).
    Non-uniform tail because w=50 has row0=249 (not 250)."""
    gs = [tuple(range(g * GRP, g * GRP + GRP)) for g in range(16)]
    gs.append((48, 49))
    gs.append((50,))
    return gs


def _build_nc(reps=1):
    import concourse.bass as bass
    import concourse.bacc as bacc
    import concourse.tile as tile
    from concourse import mybir

    f32 = mybir.dt.float32
    bf16 = mybir.dt.bfloat16

    nc = bacc.Bacc(None)
    # x_aug host layout is [B, H, C, W] bf16: window partitions are (row, chan)
    x_aug = nc.dram_tensor("x_aug", [B_LOC, H, CIN + 1, W], bf16, kind="ExternalInput")
    wblk_d = nc.dram_tensor("wblk", [KK, KDIM, NDIM], bf16, kind="ExternalInput")
    ident_d = nc.dram_tensor("ident", [MJ, MJ], bf16, kind="ExternalInput")
    y = nc.dram_tensor("y", [B_LOC, HO, WO], f32, kind="ExternalOutput")

    with tile.TileContext(nc) as tc:
        with (
            tc.tile_pool(name="consts", bufs=1) as consts,
            tc.tile_pool(name="wins", bufs=3) as wins,
            tc.tile_pool(name="stage", bufs=4) as stage,
            tc.tile_pool(name="outs", bufs=4) as outs,
            tc.tile_pool(name="cpsum", bufs=2, space="PSUM") as cpsum,
            tc.tile_pool(name="tpsum", bufs=2, space="PSUM") as tpsum,
        ):
            wblk_s = consts.tile([KDIM, KK, NDIM], bf16)
            nc.sync.dma_start(out=wblk_s[:], in_=wblk_d.rearrange("k d n -> d k n"))
            ident_s = consts.tile([MJ, MJ], bf16)
            nc.sync.dma_start(out=ident_s[:], in_=ident_d[:])

            import contextlib
            loop_ctx = tc.For_i(0, reps, 1) if reps > 1 else contextlib.nullcontext()
            with loop_ctx:
                _emit_body(nc, tc, bass, mybir, locals())
    nc.finalize()
    return nc


def _emit_body(nc, tc, bass, mybir, env):
    f32 = env["f32"]
    bf16 = env["bf16"]
    x_aug, y = env["x_aug"], env["y"]
    wblk_s, ident_s = env["wblk_s"], env["ident_s"]
    wins, stage, outs = env["wins"], env["stage"], env["outs"]
    cpsum, tpsum = env["cpsum"], env["tpsum"]
    CW = (CIN + 1) * W  # elements per image row (all channels)
    groups = _win_groups()

    for b in range(B_LOC):
        bigx = wins.tile([KDIM, N_WIN, W], bf16, name="bigx")
        # windows 0..49 (uniform row0 = 5w) in 4 chunked DMAs; w=50 alone
        x_b = x_aug[b]
        for ci, w_lo in enumerate(range(0, 50, 13)):
            w_hi = min(w_lo + 13, 50)
            nw = w_hi - w_lo
            src = bass.AP(
                tensor=x_b.tensor,
                offset=x_b.offset + 5 * w_lo * CW,
                ap=[[CW, WIN_ROWS], [W, CIN + 1], [5 * CW, nw], [1, W]],
            )
            nc.sync.dma_start(out=bigx[:, w_lo:w_hi, :], in_=src)
        nc.sync.dma_start(
            out=bigx[:, N_WIN - 1, :],
            in_=x_aug[b, HO - R:H, :, :].rearrange("r c w -> (r c) w"),
        )
        stagings = []
        for jb in range(N_JB):
            staging = stage.tile([MJ, 256], bf16, name=f"staging{jb}", tag=f"st{jb}")
            stagings.append(staging)
        for gi, grp in enumerate(groups):
            row0 = 5 * grp[0]
            for jb in range(N_JB):
                j0 = JOFF[jb]
                psum = cpsum.tile([MJ, GRP * PS_STRIDE], f32, name="psum")
                for si, w in enumerate(grp):
                    for dx in range(KK):
                        nc.tensor.matmul(
                            out=psum[:, si * PS_STRIDE:si * PS_STRIDE + NDIM],
                            lhsT=bigx[:, w, j0 + dx:j0 + dx + MJ],
                            rhs=wblk_s[:, dx, :],
                            start=(dx == 0),
                            stop=(dx == KK - 1),
                        )
                ng = len(grp)
                src = psum.rearrange("p (g s) -> p g s", s=PS_STRIDE)
                src = src[:, 0:ng, 0:NDIM].rearrange("p g (r c) -> p g r c", c=COUT)
                if ng > 1:
                    dst = stagings[jb][:, row0:row0 + ng * R].rearrange(
                        "p (g r) -> p g r", r=R)
                else:
                    # w=50: row0 = 249, not 5*50
                    dst = stagings[jb][:, HO - R:HO].rearrange(
                        "p (g r) -> p g r", r=R)
                nc.vector.tensor_reduce(
                    out=dst, in_=src,
                    axis=mybir.AxisListType.X,
                    op=mybir.AluOpType.min,
                )
        for rb in range(2):
            r0 = (0, HO - MJ)[rb]
            for jb in range(N_JB):
                ps_t = tpsum.tile([MJ, MJ], bf16, name="ps_t")
                nc.tensor.transpose(
                    out=ps_t[:], in_=stagings[jb][:, r0:r0 + MJ],
                    identity=ident_s[:],
                )
                t1 = outs.tile([MJ, MJ], bf16, name="t1")
                nc.scalar.activation(
                    out=t1[:], in_=ps_t[:],
                    func=mybir.ActivationFunctionType.Tanh,
                )
                t2 = outs.tile([MJ, MJ], f32, name="t2")
                nc.scalar.activation(
                    out=t2[:], in_=t1[:],
                    func=mybir.ActivationFunctionType.Tanh,
                )
                nc.sync.dma_start(
                    out=y[b, r0:r0 + MJ, JOFF[jb]:JOFF[jb] + MJ], in_=t2[:]
                )


def _get_compiled(reps=1):
    key = ("nc", reps)
    if key not in _cache:
        _cache[key] = _build_nc(reps)
    return _cache[key]


def build_in_maps(inputs):
    import ml_dtypes

    bf16 = ml_dtypes.bfloat16
    x = np.asarray(inputs["x"], dtype=np.float32)
    conv_weight = np.asarray(inputs["conv_weight"], dtype=np.float32)
    conv_bias = np.asarray(inputs["conv_bias"], dtype=np.float32)

    x_aug = np.empty((B, H, CIN + 1, W), dtype=bf16)
    x_aug[:, :, :CIN] = x.transpose(0, 2, 1, 3).astype(bf16)
    x_aug[:, :, CIN] = 1.0
    wblk = _build_wblocks(conv_weight, conv_bias).astype(bf16)
    ident = np.eye(MJ, dtype=bf16)

    return [
        {
            "x_aug": np.ascontiguousarray(x_aug[c * B_LOC:(c + 1) * B_LOC]),
            "wblk": wblk,
            "ident": ident,
        }
        for c in range(N_CORES)
    ]


def kernel(x, conv_weight, conv_bias):
    from concourse.bass_utils import run_bass_kernel_spmd

    in_maps = build_in_maps(
        {"x": x, "conv_weight": conv_weight, "conv_bias": conv_bias})
    nc = _get_compiled()
    res = run_bass_kernel_spmd(nc, in_maps, core_ids=list(range(N_CORES)))
    out = np.concatenate([res.results[c]["y"] for c in range(N_CORES)], axis=0)
    return out.reshape(B, 1, HO, WO)


# revision 7
# speedup vs baseline: 1.2254x; 1.0005x over previous
"""Trainium2 Bass kernel for: conv2d(16->64, 3x3, VALID) + bias -> min over
channels -> tanh(tanh()).  Input x [64,16,256,256] f32, output [64,1,254,254].

Strategy (per core, data-parallel over batch: 8 images/core):
  - Conv as matmuls with the *x-patch as the stationary operand* (bf16) and a
    block-Toeplitz weight matrix as the moving operand, so conv output lands
    as [width-positions (partitions), rows*couts (free)] -> channel-min is a
    free-dim DVE reduce_min.
  - Contraction K = 17 channels x 7 rows = 119 (channel 16 is a host-added
    ones-channel carrying the bias).  A 7-row window yields R=5 output rows;
    the 3 kernel x-taps are 3 PSUM-accumulated matmuls whose stationary
    operands are the same SBUF window tile sliced at column offset dx.
  - bf16 stationaries are 128 columns wide -> compiler enables FWL (2x faster
    LDWEIGHTS, hidden under the 320-col matmuls).
  - DVE reduce_min batches THREE windows per instruction (psum groups at
    512-elem bank strides) to amortize the 120-cycle PSUM-source overhead:
    in [128, 3, 5, 64] -> out [128, 15] staging rows.
  - Epilogue: PE transposes 128x128 chunks of the [j, row] staging; double
    tanh on ScalarE; DMA out.  j/row blocks at offsets {0, 126} overlap by 2
    (identical values) to cover 254 with 128-wide tiles.
"""

import sys

for _p in ("/opt/trn_rl_repo", "/root/.axon_site/_ro/trn_rl_repo"):
    if _p not in sys.path:
        sys.path.insert(0, _p)

import numpy as np

B, CIN, H, W = 64, 16, 256, 256
COUT, KK = 64, 3
HO, WO = H - 2, W - 2  # 254
N_CORES = 8
B_LOC = B // N_CORES  # 8 images per core

# geometry
WIN_ROWS = 7           # input rows per window
R = WIN_ROWS - KK + 1  # 5 output rows per window
KDIM = (CIN + 1) * WIN_ROWS  # 119 real contraction rows (incl. ones channel)
KPAD = 128             # padded contraction: rows 119..127 zero (enables FWL)
NDIM = R * COUT        # 320 moving free size
MJ = 128               # output width positions per j-block (FWL needs 128)
JOFF = (0, WO - MJ)    # j-block origins: {0, 126}, overlap 2 cols
N_JB = 2
N_WIN = 51             # windows: row0 = 5w for w<50, 249 for w=50
GRP = 3                # windows per DVE reduce
PS_STRIDE = 512        # f32 elems between psum groups (= one 2KB bank)

_cache = {}


def _build_wblocks(conv_weight, conv_bias):
    """wblk[dx][rho*17+ci, r*64+co] = W[co,ci,rho-r,dx]; bias on the ones-
    channel row (rho=0, ci=CIN) of dx=0.  Partition order matches the
    [B, H, C, W] host layout of x so the window DMA merges (row, chan)."""
    wblk = np.zeros((KK, KPAD, NDIM), dtype=np.float32)
    for dx in range(KK):
        for ci in range(CIN):
            for rho in range(WIN_ROWS):
                k = rho * (CIN + 1) + ci
                for r in range(R):
                    dy = rho - r
                    if 0 <= dy < KK:
                        wblk[dx, k, r * COUT:(r + 1) * COUT] = conv_weight[:, ci, dy, dx]
    k_bias = CIN  # (rho=0, ci=16)
    for r in range(R):
        wblk[0, k_bias, r * COUT:(r + 1) * COUT] = conv_bias
    return wblk


def _win_groups():
    """Reduce groups: 16 triples (w 0..47), the pair (48,49), single (50).
    Non-uniform tail because w=50 has row0=249 (not 250)."""
    gs = [tuple(range(g * GRP, g * GRP + GRP)) for g in range(16)]
    gs.append((48, 49))
    gs.append((50,))
    return gs


def _build_nc(reps=1):
    import concourse.bass as bass
    import concourse.bacc as bacc
    import concourse.tile as tile
    from concourse import mybir

    f32 = mybir.dt.float32
    bf16 = mybir.dt.bfloat16

    nc = bacc.Bacc(None)
    # x_aug host layout is [B, H, C, W] bf16: window partitions are (row, chan)
    x_aug = nc.dram_tensor("x_aug", [B_LOC, H, CIN + 1, W], bf16, kind="ExternalInput")
    wblk_d = nc.dram_tensor("wblk", [KK, KPAD, NDIM], bf16, kind="ExternalInput")
    ident_d = nc.dram_tensor("ident", [MJ, MJ], bf16, kind="ExternalInput")
    y = nc.dram_tensor("y", [B_LOC, HO, WO], f32, kind="ExternalOutput")

    with tile.TileContext(nc) as tc:
        with (
            tc.tile_pool(name="consts", bufs=1) as consts,
            tc.tile_pool(name="wins", bufs=3) as wins,
            tc.tile_pool(name="stage", bufs=4) as stage,
            tc.tile_pool(name="outs", bufs=4) as outs,
            tc.tile_pool(name="cpsum", bufs=2, space="PSUM") as cpsum,
            tc.tile_pool(name="tpsum", bufs=2, space="PSUM") as tpsum,
        ):
            wblk_s = consts.tile([KPAD, KK, NDIM], bf16)
            nc.sync.dma_start(out=wblk_s[:], in_=wblk_d.rearrange("k d n -> d k n"))
            ident_s = consts.tile([MJ, MJ], bf16)
            nc.sync.dma_start(out=ident_s[:], in_=ident_d[:])

            for t in range(3):
                bx = wins.tile([KPAD, N_WIN, W], bf16, name=f"bigx{t}",
                               tag=f"bigx{t}", bufs=1)
                nc.vector.memset(bx[96:KPAD, :, :], 0.0)

            import contextlib
            loop_ctx = tc.For_i(0, reps, 1) if reps > 1 else contextlib.nullcontext()
            with loop_ctx:
                _emit_body(nc, tc, bass, mybir, locals())
    nc.finalize()
    return nc


def _emit_body(nc, tc, bass, mybir, env):
    f32 = env["f32"]
    bf16 = env["bf16"]
    x_aug, y = env["x_aug"], env["y"]
    wblk_s, ident_s = env["wblk_s"], env["ident_s"]
    wins, stage, outs = env["wins"], env["stage"], env["outs"]
    cpsum, tpsum = env["cpsum"], env["tpsum"]
    CW = (CIN + 1) * W  # elements per image row (all channels)
    groups = _win_groups()

    for b in range(B_LOC):
        bigx = wins.tile([KPAD, N_WIN, W], bf16, name=f"bigx{b % 3}",
                         tag=f"bigx{b % 3}", bufs=1)
        # windows 0..49 (uniform row0 = 5w) in 4 chunked DMAs; w=50 alone
        x_b = x_aug[b]
        for ci, w_lo in enumerate(range(0, 50, 13)):
            w_hi = min(w_lo + 13, 50)
            nw = w_hi - w_lo
            src = bass.AP(
                tensor=x_b.tensor,
                offset=x_b.offset + 5 * w_lo * CW,
                ap=[[CW, WIN_ROWS], [W, CIN + 1], [5 * CW, nw], [1, W]],
            )
            nc.sync.dma_start(out=bigx[0:KDIM, w_lo:w_hi, :], in_=src)
        nc.sync.dma_start(
            out=bigx[0:KDIM, N_WIN - 1, :],
            in_=x_aug[b, HO - R:H, :, :].rearrange("r c w -> (r c) w"),
        )
        stagings = []
        for jb in range(N_JB):
            staging = stage.tile([MJ, 256], bf16, name=f"staging{jb}", tag=f"st{jb}")
            stagings.append(staging)
        for gi, grp in enumerate(groups):
            row0 = 5 * grp[0]
            for jb in range(N_JB):
                j0 = JOFF[jb]
                psum = cpsum.tile([MJ, GRP * PS_STRIDE], f32, name="psum")
                for si, w in enumerate(grp):
                    for dx in range(KK):
                        nc.tensor.matmul(
                            out=psum[:, si * PS_STRIDE:si * PS_STRIDE + NDIM],
                            lhsT=bigx[:, w, j0 + dx:j0 + dx + MJ],
                            rhs=wblk_s[:, dx, :],
                            start=(dx == 0),
                            stop=(dx == KK - 1),
                        )
                ng = len(grp)
                src = psum.rearrange("p (g s) -> p g s", s=PS_STRIDE)
                src = src[:, 0:ng, 0:NDIM].rearrange("p g (r c) -> p g r c", c=COUT)
                if ng > 1:
                    dst = stagings[jb][:, row0:row0 + ng * R].rearrange(
                        "p (g r) -> p g r", r=R)
                else:
                    # w=50: row0 = 249, not 5*50
                    dst = stagings[jb][:, HO - R:HO].rearrange(
                        "p (g r) -> p g r", r=R)
                nc.vector.tensor_reduce(
                    out=dst, in_=src,
                    axis=mybir.AxisListType.X,
                    op=mybir.AluOpType.min,
                )
        for rb in range(2):
            r0 = (0, HO - MJ)[rb]
            for jb in range(N_JB):
                ps_t = tpsum.tile([MJ, MJ], bf16, name="ps_t")
                nc.tensor.transpose(
                    out=ps_t[:], in_=stagings[jb][:, r0:r0 + MJ],
                    identity=ident_s[:],
                )
                t1 = outs.tile([MJ, MJ], bf16, name="t1")
                nc.scalar.activation(
                    out=t1[:], in_=ps_t[:],
                    func=mybir.ActivationFunctionType.Tanh,
                )
                t2 = outs.tile([MJ, MJ], f32, name="t2")
                nc.scalar.activation(
                    out=t2[:], in_=t1[:],
                    func=mybir.ActivationFunctionType.Tanh,
                )
                nc.sync.dma_start(
                    out=y[b, r0:r0 + MJ, JOFF[jb]:JOFF[jb] + MJ], in_=t2[:]
                )


def _get_compiled(reps=1):
    key = ("nc", reps)
    if key not in _cache:
        _cache[key] = _build_nc(reps)
    return _cache[key]


def build_in_maps(inputs):
    import ml_dtypes

    bf16 = ml_dtypes.bfloat16
    x = np.asarray(inputs["x"], dtype=np.float32)
    conv_weight = np.asarray(inputs["conv_weight"], dtype=np.float32)
    conv_bias = np.asarray(inputs["conv_bias"], dtype=np.float32)

    x_aug = np.empty((B, H, CIN + 1, W), dtype=bf16)
    x_aug[:, :, :CIN] = x.transpose(0, 2, 1, 3).astype(bf16)
    x_aug[:, :, CIN] = 1.0
    wblk = _build_wblocks(conv_weight, conv_bias).astype(bf16)
    ident = np.eye(MJ, dtype=bf16)

    return [
        {
            "x_aug": np.ascontiguousarray(x_aug[c * B_LOC:(c + 1) * B_LOC]),
            "wblk": wblk,
            "ident": ident,
        }
        for c in range(N_CORES)
    ]


def kernel(x, conv_weight, conv_bias):
    from concourse.bass_utils import run_bass_kernel_spmd

    in_maps = build_in_maps(
        {"x": x, "conv_weight": conv_weight, "conv_bias": conv_bias})
    nc = _get_compiled()
    res = run_bass_kernel_spmd(nc, in_maps, core_ids=list(range(N_CORES)))
    out = np.concatenate([res.results[c]["y"] for c in range(N_CORES)], axis=0)
    return out.reshape(B, 1, HO, WO)
